# revision 21
# baseline (speedup 1.0000x reference)
"""Multi-head attention (B=4, S=2048, D=1024, H=16) on 8 NeuronCores.

Reference quirk: the key-padding mask uses jnp.tile(valid_length, H) indexed
by the flat (b*H + h) head-batch index, so the effective mask length for
(batch b, head h) is valid_length[h % 4] -- it depends on the head CLASS
(h mod 4), not the batch.

Sharding: core i handles batch i%4 and the 8 heads {4P..4P+3, 4P+8..4P+11}
(P = i//4).  Those 8 heads contain each mask class exactly twice, so every
core does identical work (load-balanced by construction), and key/value work
beyond valid_length[class] (rounded up to 128) is skipped entirely.  The two
same-class heads (h, h+8) are row-packed into one 64-contraction PE pair.
Per-core partial outputs (rank-512 contributions through Wo) are summed on
the host (cores i and i+4 hold the two halves of batch i%4's heads).

All matmuls run in bf16 (fp32 PSUM accumulation).  Attention is computed in
"transposed" orientation S^T[k, q] so that softmax masking is a per-partition
exp bias, the k-sum comes free via an appended ones-column on V, and no
on-chip transposes are needed anywhere.
"""

import sys

for _p in ("/opt/trn_rl_repo", "/root/.axon_site/_ro/trn_rl_repo"):
    if _p not in sys.path:
        sys.path.insert(0, _p)

import numpy as np
import ml_dtypes

B, S, D, H = 4, 2048, 1024, 16
HD = D // H  # 64
NCORES = 8
NSLOT = 4  # head classes (h % 4) per core, 2 heads each
KT = 128  # k-tile size
QB = 512  # q block
MASK_BIAS = -30000.0  # exp(s/8 + bias) == 0 for masked rows (s/8 is O(10))

_compiled = {}  # (T0,T1,T2,T3) -> compiled nc


def core_heads(core):
    """The 8 heads of `core`, in (slot, pair) order: [hA0, hB0, hA1, ...]."""
    P = core // 4
    heads = []
    for c in range(NSLOT):
        heads += [c + 4 * P, c + 8 + 4 * P]
    return heads


def _build(Ts, taps=False):
    """Build + compile the single SPMD program for k-tile class profile Ts."""
    import concourse.bacc as bacc
    import concourse.tile as tile
    import concourse.mybir as mybir

    fp32 = mybir.dt.float32
    bf16 = mybir.dt.bfloat16
    fp16 = mybir.dt.float16
    EXP = mybir.ActivationFunctionType.Exp

    CKMAX = max(Ts) * KT
    DT = D // 128  # 8 contraction tiles for the projections
    NQ = S // QB  # 4 q blocks
    HPC2 = 2 * NSLOT * HD  # 512 head-dim columns per core

    nc = bacc.Bacc("TRN2", target_bir_lowering=False, debug=False, num_devices=NCORES)

    qT = nc.dram_tensor("qT", [D, S], bf16, kind="ExternalInput")
    kT = nc.dram_tensor("kT", [D, CKMAX], bf16, kind="ExternalInput")
    vT = nc.dram_tensor("vT", [D, CKMAX], bf16, kind="ExternalInput")
    wq = nc.dram_tensor("wq", [D, HPC2], bf16, kind="ExternalInput")
    wk = nc.dram_tensor("wk", [D, HPC2], bf16, kind="ExternalInput")
    wv = nc.dram_tensor("wv", [D, HPC2], bf16, kind="ExternalInput")
    wo = nc.dram_tensor("wo", [HPC2, D], bf16, kind="ExternalInput")
    bias_in = nc.dram_tensor("bias", [KT, NSLOT], fp32, kind="ExternalInput")
    out2 = nc.dram_tensor("out2", [S, D], fp16, kind="ExternalOutput")
    if taps:
        dbg_qts = nc.dram_tensor("dbg_qts", [NSLOT, 128, S], bf16, kind="ExternalOutput")
        dbg_kts = nc.dram_tensor("dbg_kts", [NSLOT, 128, CKMAX], bf16, kind="ExternalOutput")
        dbg_ve = nc.dram_tensor(
            "dbg_ve", [NSLOT, 128, max(Ts), 2, HD + 1], bf16, kind="ExternalOutput"
        )
        dbg_p = nc.dram_tensor("dbg_p", [128, 2, max(Ts), QB], bf16, kind="ExternalOutput")
        dbg_at = nc.dram_tensor("dbg_at", [NSLOT, 128, S], bf16, kind="ExternalOutput")

    with tile.TileContext(nc) as tc:
        with (
            tc.tile_pool(name="w", bufs=1) as wpool,
            tc.tile_pool(name="x", bufs=2) as xpool,
            tc.tile_pool(name="qk", bufs=1) as qkpool,
            tc.tile_pool(name="sm", bufs=2) as smpool,
            tc.tile_pool(name="o", bufs=2) as opool,
            tc.tile_pool(name="psmm", bufs=2, space="PSUM") as psmm,
            tc.tile_pool(name="pss", bufs=2, space="PSUM") as pss,
            tc.tile_pool(name="pspv", bufs=2, space="PSUM") as pspv,
        ):
            # ---- persistent weights ----
            wq_sb = wpool.tile([128, DT, HPC2], bf16, tag="wq")
            wk_sb = wpool.tile([128, DT, HPC2], bf16, tag="wk")
            wv_sb = wpool.tile([128, DT, HPC2], bf16, tag="wv")
            wo_sb = wpool.tile([128, NSLOT, D], bf16, tag="wo")
            bias_sb = wpool.tile([KT, NSLOT], fp32, tag="bias")
            nc.sync.dma_start(wq_sb[:], wq.ap().rearrange("(t p) c -> p t c", p=128))
            nc.sync.dma_start(wk_sb[:], wk.ap().rearrange("(t p) c -> p t c", p=128))
            nc.sync.dma_start(wv_sb[:], wv.ap().rearrange("(t p) c -> p t c", p=128))
            nc.sync.dma_start(wo_sb[:], wo.ap().rearrange("(c p) n -> p c n", p=128))
            nc.sync.dma_start(bias_sb[:], bias_in.ap())

            # ---- projections (slot s uses weight columns [128s : 128s+128]) ----
            qts = [
                qkpool.tile([128, S], bf16, tag=f"qts{s}", name=f"qts{s}")
                for s in range(NSLOT)
            ]
            xq = xpool.tile([128, DT, S], bf16, tag="x", name="xq")
            nc.sync.dma_start(xq[:], qT.ap().rearrange("(t p) q -> p t q", p=128))
            for s in range(NSLOT):
                csl = slice(s * 128, (s + 1) * 128)
                for qb in range(NQ):
                    ps = psmm.tile([128, QB], fp32, tag="mm", name="psq")
                    for dt in range(DT):
                        nc.tensor.matmul(
                            ps[:],
                            wq_sb[:, dt, csl],
                            xq[:, dt, qb * QB : (qb + 1) * QB],
                            start=(dt == 0),
                            stop=(dt == DT - 1),
                        )
                    nc.vector.tensor_copy(qts[s][:, qb * QB : (qb + 1) * QB], ps[:])

            kts = [
                qkpool.tile([128, Ts[s] * KT], bf16, tag=f"kts{s}", name=f"kts{s}")
                for s in range(NSLOT)
            ]
            xk = xpool.tile([128, DT, CKMAX], bf16, tag="x", name="xk")
            nc.sync.dma_start(xk[:], kT.ap().rearrange("(t p) k -> p t k", p=128))
            for s in range(NSLOT):
                csl = slice(s * 128, (s + 1) * 128)
                CK = Ts[s] * KT
                for k0 in range(0, CK, QB):
                    kw = min(QB, CK - k0)
                    ps = psmm.tile([128, QB], fp32, tag="mm", name="psk")
                    for dt in range(DT):
                        nc.tensor.matmul(
                            ps[:, :kw],
                            wk_sb[:, dt, csl],
                            xk[:, dt, k0 : k0 + kw],
                            start=(dt == 0),
                            stop=(dt == DT - 1),
                        )
                    nc.vector.tensor_copy(kts[s][:, k0 : k0 + kw], ps[:, :kw])

            # V_ext: [128k, T, 2 heads, 65] with ones in column 64
            ve = [
                qkpool.tile([128, Ts[s], 2, HD + 1], bf16, tag=f"ve{s}", name=f"ve{s}")
                for s in range(NSLOT)
            ]
            xv = xpool.tile([128, DT, CKMAX], bf16, tag="x", name="xv")
            nc.sync.dma_start(xv[:], vT.ap().rearrange("(t p) k -> p t k", p=128))
            for s in range(NSLOT):
                csl = slice(s * 128, (s + 1) * 128)
                for kt in range(Ts[s]):
                    nc.gpsimd.memset(ve[s][:, kt, :, HD : HD + 1], 1.0)
                    ps = psmm.tile([128, QB], fp32, tag="mm", name="psv")
                    for dt in range(DT):
                        nc.tensor.matmul(
                            ps[:, 0:128],
                            xv[:, dt, kt * KT : (kt + 1) * KT],
                            wv_sb[:, dt, csl],
                            start=(dt == 0),
                            stop=(dt == DT - 1),
                        )
                    nc.vector.tensor_copy(
                        ve[s][:, kt, :, 0:HD],
                        ps[:, 0:128].rearrange("p (h d) -> p h d", h=2),
                    )

            if taps:
                for s in range(NSLOT):
                    nc.sync.dma_start(dbg_qts.ap()[s], qts[s][:])
                    nc.sync.dma_start(dbg_kts.ap()[s, :, 0 : Ts[s] * KT], kts[s][:])
                    nc.sync.dma_start(dbg_ve.ap()[s, :, 0 : Ts[s]], ve[s][:])

            # ---- attention (slot s == mask class s) ----
            aT = [
                qkpool.tile([128, S], bf16, tag=f"at{s}", name=f"at{s}")
                for s in range(NSLOT)
            ]
            for s in range(NSLOT):
                T = Ts[s]
                for qb in range(NQ):
                    qsl = slice(qb * QB, (qb + 1) * QB)
                    p = xpool.tile([128, 2, T, QB], bf16, tag="x", name="p")
                    for kt in range(T):
                        ksl = slice(kt * KT, (kt + 1) * KT)
                        ss = pss.tile([128, 2, QB], fp32, tag="s", name="ss")
                        # scores^T, 2 same-class heads packed as PE row tiles
                        nc.tensor.matmul(ss[:, 0, :], kts[s][0:64, ksl], qts[s][0:64, qsl])
                        nc.tensor.matmul(
                            ss[:, 1, :], kts[s][64:128, ksl], qts[s][64:128, qsl]
                        )
                        bias_ap = bias_sb[:, s : s + 1] if kt == T - 1 else 0.0
                        nc.scalar.activation(
                            p[:, :, kt, :], ss[:], EXP, bias=bias_ap, scale=0.125
                        )
                    if taps and s == 0:
                        if qb == 0:
                            nc.sync.dma_start(dbg_p.ap()[:, :, 0:T, :], p[:])
                    pv = [
                        pspv.tile([128, QB], fp32, tag="pv", name=f"pv{h}")
                        for h in range(2)
                    ]
                    for h in range(2):
                        for kt in range(T):
                            nc.tensor.matmul(
                                pv[h][0 : HD + 1, :],
                                ve[s][:, kt, h, :],
                                p[:, h, kt, :],
                                start=(kt == 0),
                                stop=(kt == T - 1),
                            )
                    # normalize: aT[s][h*64:(h+1)*64, qsl] = pv[h][:64] / pv[h][64]
                    # (sum row moved to partition 0 first: reciprocal /
                    #  partition_broadcast only work from base partition 0)
                    s_sb = smpool.tile([HD + 1, 2, QB], fp32, tag="ssb", name="ssb")
                    for h in range(2):
                        nc.vector.tensor_copy(
                            s_sb[HD : HD + 1, h, :], pv[h][HD : HD + 1, :]
                        )
                    s0 = smpool.tile([1, 2, QB], fp32, tag="s0", name="s0")
                    nc.sync.dma_start(s0[:], s_sb[HD : HD + 1, :, :])
                    r0 = smpool.tile([1, 2, QB], fp32, tag="r0", name="r0")
                    nc.vector.reciprocal(r0[:], s0[:])
                    for h in range(2):
                        rb = smpool.tile([HD, QB], fp32, name=f"rb{h}", tag="rb")
                        nc.gpsimd.partition_broadcast(rb[:], r0[0:1, h, :])
                        if h == 0:
                            nc.vector.tensor_mul(
                                aT[s][0:HD, qsl], pv[h][0:HD, :], rb[:]
                            )
                        else:
                            tmp = smpool.tile([HD, QB], bf16, tag="tmp", name="tmp")
                            nc.vector.tensor_mul(tmp[:], pv[h][0:HD, :], rb[:])
                            nc.sync.dma_start(aT[s][HD:128, qsl], tmp[:])

            if taps:
                for s in range(NSLOT):
                    nc.sync.dma_start(dbg_at.ap()[s], aT[s][:])

            # ---- Wo:  out2 = sum_s aT[s].T @ wo[s]  (partial, fp16) ----
            for qt in range(S // 128):
                ob = opool.tile([128, D], fp16, tag="ob", name="ob")
                for nh in range(2):
                    nsl = slice(nh * 512, (nh + 1) * 512)
                    ps = psmm.tile([128, QB], fp32, tag="mm", name="pso")
                    for s in range(NSLOT):
                        nc.tensor.matmul(
                            ps[:],
                            aT[s][:, qt * 128 : (qt + 1) * 128],
                            wo_sb[:, s, nsl],
                            start=(s == 0),
                            stop=(s == NSLOT - 1),
                        )
                    if nh == 0:
                        nc.vector.tensor_copy(ob[:, nsl], ps[:])
                    else:
                        nc.scalar.copy(ob[:, nsl], ps[:])
                nc.sync.dma_start(out2.ap()[qt * 128 : (qt + 1) * 128, :], ob[:])

    nc.compile()
    return nc


def build_in_maps(query, key, value, valid_length, Wq, Wk, Wv, Wo):
    """Host-side sharding. Returns (Ts, in_maps)."""
    valid = np.asarray(valid_length).astype(np.int64)
    Ts = tuple(int(-(-v // KT)) for v in valid)
    CKMAX = max(Ts) * KT

    bf = ml_dtypes.bfloat16
    query = np.asarray(query)
    key = np.asarray(key)
    value = np.asarray(value)
    qTs = [np.ascontiguousarray(query[b].T).astype(bf) for b in range(B)]
    kTs = [np.ascontiguousarray(key[b].T[:, :CKMAX]).astype(bf) for b in range(B)]
    vTs = [np.ascontiguousarray(value[b].T[:, :CKMAX]).astype(bf) for b in range(B)]

    bias = np.zeros((KT, NSLOT), np.float32)
    for s in range(NSLOT):
        rem = int(valid[s]) - (Ts[s] - 1) * KT  # 1..128 valid rows in last tile
        bias[rem:, s] = MASK_BIAS

    Wqb = np.asarray(Wq).astype(bf)
    Wkb = np.asarray(Wk).astype(bf)
    Wvb = np.asarray(Wv).astype(bf)
    Wob = np.asarray(Wo).astype(bf)

    in_maps = []
    for c in range(NCORES):
        beta = c % 4
        hcols = np.concatenate(
            [np.arange(h * HD, (h + 1) * HD) for h in core_heads(c)]
        )
        in_maps.append(
            {
                "qT": qTs[beta],
                "kT": kTs[beta],
                "vT": vTs[beta],
                "wq": np.ascontiguousarray(Wqb[:, hcols]),
                "wk": np.ascontiguousarray(Wkb[:, hcols]),
                "wv": np.ascontiguousarray(Wvb[:, hcols]),
                "wo": np.ascontiguousarray(Wob[hcols, :]),
                "bias": bias,
            }
        )
    return Ts, in_maps


def kernel(query, key, value, valid_length, Wq, Wk, Wv, Wo):
    from concourse.bass_utils import run_bass_kernel_spmd

    Ts, in_maps = build_in_maps(
        query, key, value, valid_length, Wq, Wk, Wv, Wo
    )
    if Ts not in _compiled:
        _compiled[Ts] = _build(Ts)
    nc = _compiled[Ts]

    res = run_bass_kernel_spmd(nc, in_maps, list(range(NCORES)))
    out = np.zeros((B, S, D), np.float32)
    for c in range(NCORES):
        out[c % 4] += res.results[c]["out2"].astype(np.float32)
    return out


# revision 24
# speedup vs baseline: 1.0366x; 1.0366x over previous
"""Multi-head attention (B=4, S=2048, D=1024, H=16) on 8 NeuronCores.

Reference quirk: the key-padding mask uses jnp.tile(valid_length, H) indexed
by the flat (b*H + h) head-batch index, so the effective mask length for
(batch b, head h) is valid_length[h % 4] -- it depends on the head CLASS
(h mod 4), not the batch.

Sharding: core i handles batch i%4 and the 8 heads {4P..4P+3, 4P+8..4P+11}
(P = i//4).  Those 8 heads contain each mask class exactly twice, so every
core does identical work (load-balanced by construction), and key/value work
beyond valid_length[class] (rounded up to 128) is skipped entirely.  The two
same-class heads (h, h+8) are row-packed into one 64-contraction PE pair.
Per-core partial outputs (rank-512 contributions through Wo) are summed on
the host (cores i and i+4 hold the two halves of batch i%4's heads).

All matmuls run in bf16 (fp32 PSUM accumulation).  Attention is computed in
"transposed" orientation S^T[k, q] so that softmax masking is a per-partition
exp bias, the k-sum comes free via an appended ones-column on V, and no
on-chip transposes are needed anywhere.
"""

import sys

for _p in ("/opt/trn_rl_repo", "/root/.axon_site/_ro/trn_rl_repo"):
    if _p not in sys.path:
        sys.path.insert(0, _p)

import numpy as np
import ml_dtypes

B, S, D, H = 4, 2048, 1024, 16
HD = D // H  # 64
NCORES = 8
NSLOT = 4  # head classes (h % 4) per core, 2 heads each
KT = 128  # k-tile size
QB = 512  # q block
MASK_BIAS = -30000.0  # exp(s/8 + bias) == 0 for masked rows (s/8 is O(10))

_compiled = {}  # (T0,T1,T2,T3) -> compiled nc


def core_heads(core):
    """The 8 heads of `core`, in (slot, pair) order: [hA0, hB0, hA1, ...]."""
    P = core // 4
    heads = []
    for c in range(NSLOT):
        heads += [c + 4 * P, c + 8 + 4 * P]
    return heads


def _build(Ts, taps=False, bench_iters=0):
    """Build + compile the single SPMD program for k-tile class profile Ts.

    bench_iters > 0 wraps the whole body in a hardware loop for timing.
    """
    import contextlib
    import concourse.bacc as bacc
    import concourse.tile as tile
    import concourse.mybir as mybir

    fp32 = mybir.dt.float32
    bf16 = mybir.dt.bfloat16
    fp16 = mybir.dt.float16
    EXP = mybir.ActivationFunctionType.Exp

    CKMAX = max(Ts) * KT
    DT = D // 128  # 8 contraction tiles for the projections
    NQ = S // QB  # 4 q blocks
    HPC2 = 2 * NSLOT * HD  # 512 head-dim columns per core

    nc = bacc.Bacc("TRN2", target_bir_lowering=False, debug=False, num_devices=NCORES)

    qT = nc.dram_tensor("qT", [D, S], bf16, kind="ExternalInput")
    kT = nc.dram_tensor("kT", [D, CKMAX], bf16, kind="ExternalInput")
    vT = nc.dram_tensor("vT", [D, CKMAX], bf16, kind="ExternalInput")
    wq = nc.dram_tensor("wq", [D, HPC2], bf16, kind="ExternalInput")
    wk = nc.dram_tensor("wk", [D, HPC2], bf16, kind="ExternalInput")
    wv = nc.dram_tensor("wv", [D, HPC2], bf16, kind="ExternalInput")
    wo = nc.dram_tensor("wo", [HPC2, D], bf16, kind="ExternalInput")
    bias_in = nc.dram_tensor("bias", [KT, NSLOT], fp32, kind="ExternalInput")
    out2 = nc.dram_tensor("out2", [S, D], fp16, kind="ExternalOutput")
    if taps:
        dbg_qts = nc.dram_tensor("dbg_qts", [NSLOT, 128, S], bf16, kind="ExternalOutput")
        dbg_kts = nc.dram_tensor("dbg_kts", [NSLOT, 128, CKMAX], bf16, kind="ExternalOutput")
        dbg_ve = nc.dram_tensor(
            "dbg_ve", [NSLOT, 128, max(Ts), 2, HD + 1], bf16, kind="ExternalOutput"
        )
        dbg_p = nc.dram_tensor("dbg_p", [128, 2, max(Ts), QB], bf16, kind="ExternalOutput")
        dbg_at = nc.dram_tensor("dbg_at", [NSLOT, 128, S], bf16, kind="ExternalOutput")

    with tile.TileContext(nc) as tc:
        with (
            tc.tile_pool(name="w", bufs=1) as wpool,
            tc.tile_pool(name="x", bufs=2) as xpool,
            tc.tile_pool(name="qk", bufs=1) as qkpool,
            tc.tile_pool(name="sm", bufs=2) as smpool,
            tc.tile_pool(name="o", bufs=2) as opool,
            tc.tile_pool(name="psmm", bufs=2, space="PSUM") as psmm,
            tc.tile_pool(name="pss", bufs=2, space="PSUM") as pss,
            tc.tile_pool(name="pspv", bufs=2, space="PSUM") as pspv,
        ):
            # ---- persistent weights ----
            wq_sb = wpool.tile([128, DT, HPC2], bf16, tag="wq")
            wk_sb = wpool.tile([128, DT, HPC2], bf16, tag="wk")
            wv_sb = wpool.tile([128, DT, HPC2], bf16, tag="wv")
            wo_sb = wpool.tile([128, NSLOT, D], bf16, tag="wo")
            bias_sb = wpool.tile([KT, NSLOT], fp32, tag="bias")
            nc.sync.dma_start(wq_sb[:], wq.ap().rearrange("(t p) c -> p t c", p=128))
            nc.sync.dma_start(wk_sb[:], wk.ap().rearrange("(t p) c -> p t c", p=128))
            nc.sync.dma_start(wv_sb[:], wv.ap().rearrange("(t p) c -> p t c", p=128))
            nc.sync.dma_start(wo_sb[:], wo.ap().rearrange("(c p) n -> p c n", p=128))
            nc.sync.dma_start(bias_sb[:], bias_in.ap())

            loop_cm = (
                tc.For_i(0, bench_iters, 1)
                if bench_iters > 0
                else contextlib.nullcontext()
            )
            with loop_cm:
                _emit_body(nc, tc, locals())

    nc.compile()
    return nc


def _emit_body(nc, tc, env):
    import concourse.mybir as mybir

    fp32 = mybir.dt.float32
    bf16 = mybir.dt.bfloat16
    fp16 = mybir.dt.float16
    EXP = mybir.ActivationFunctionType.Exp
    Ts, taps = env["Ts"], env["taps"]
    DT, NQ = env["DT"], env["NQ"]
    CKMAX = env["CKMAX"]
    qT, kT, vT, out2 = env["qT"], env["kT"], env["vT"], env["out2"]
    wq_sb, wk_sb, wv_sb, wo_sb = env["wq_sb"], env["wk_sb"], env["wv_sb"], env["wo_sb"]
    bias_sb = env["bias_sb"]
    xpool, qkpool, smpool, opool = env["xpool"], env["qkpool"], env["smpool"], env["o" "pool"]
    psmm, pss, pspv = env["psmm"], env["pss"], env["pspv"]
    if taps:
        dbg_qts, dbg_kts, dbg_ve = env["dbg_qts"], env["dbg_kts"], env["dbg_ve"]
        dbg_p, dbg_at = env["dbg_p"], env["dbg_at"]

    if True:
        if True:
            # ---- projections (slot s uses weight columns [128s : 128s+128]) ----
            qts = [
                qkpool.tile([128, S], bf16, tag=f"qts{s}", name=f"qts{s}")
                for s in range(NSLOT)
            ]
            xq = xpool.tile([128, DT, S], bf16, tag="x", name="xq")
            nc.sync.dma_start(xq[:], qT.ap().rearrange("(t p) q -> p t q", p=128))
            for s in range(NSLOT):
                csl = slice(s * 128, (s + 1) * 128)
                for qb in range(NQ):
                    ps = psmm.tile([128, QB], fp32, tag="mm", name="psq")
                    for dt in range(DT):
                        nc.tensor.matmul(
                            ps[:],
                            wq_sb[:, dt, csl],
                            xq[:, dt, qb * QB : (qb + 1) * QB],
                            start=(dt == 0),
                            stop=(dt == DT - 1),
                        )
                    nc.vector.tensor_copy(qts[s][:, qb * QB : (qb + 1) * QB], ps[:])

            kts = [
                qkpool.tile([128, Ts[s] * KT], bf16, tag=f"kts{s}", name=f"kts{s}")
                for s in range(NSLOT)
            ]
            xk = xpool.tile([128, DT, CKMAX], bf16, tag="x", name="xk")
            nc.sync.dma_start(xk[:], kT.ap().rearrange("(t p) k -> p t k", p=128))
            for s in range(NSLOT):
                csl = slice(s * 128, (s + 1) * 128)
                CK = Ts[s] * KT
                for k0 in range(0, CK, QB):
                    kw = min(QB, CK - k0)
                    ps = psmm.tile([128, QB], fp32, tag="mm", name="psk")
                    for dt in range(DT):
                        nc.tensor.matmul(
                            ps[:, :kw],
                            wk_sb[:, dt, csl],
                            xk[:, dt, k0 : k0 + kw],
                            start=(dt == 0),
                            stop=(dt == DT - 1),
                        )
                    nc.vector.tensor_copy(kts[s][:, k0 : k0 + kw], ps[:, :kw])

            # V_ext: [128k, T, 2 heads, 65] with ones in column 64
            ve = [
                qkpool.tile([128, Ts[s], 2, HD + 1], bf16, tag=f"ve{s}", name=f"ve{s}")
                for s in range(NSLOT)
            ]
            xv = xpool.tile([128, DT, CKMAX], bf16, tag="x", name="xv")
            nc.sync.dma_start(xv[:], vT.ap().rearrange("(t p) k -> p t k", p=128))
            for s in range(NSLOT):
                csl = slice(s * 128, (s + 1) * 128)
                for kt in range(Ts[s]):
                    nc.gpsimd.memset(ve[s][:, kt, :, HD : HD + 1], 1.0)
                    ps = psmm.tile([128, QB], fp32, tag="mm", name="psv")
                    for dt in range(DT):
                        nc.tensor.matmul(
                            ps[:, 0:128],
                            xv[:, dt, kt * KT : (kt + 1) * KT],
                            wv_sb[:, dt, csl],
                            start=(dt == 0),
                            stop=(dt == DT - 1),
                        )
                    nc.vector.tensor_copy(
                        ve[s][:, kt, :, 0:HD],
                        ps[:, 0:128].rearrange("p (h d) -> p h d", h=2),
                    )

            if taps:
                for s in range(NSLOT):
                    nc.sync.dma_start(dbg_qts.ap()[s], qts[s][:])
                    nc.sync.dma_start(dbg_kts.ap()[s, :, 0 : Ts[s] * KT], kts[s][:])
                    nc.sync.dma_start(dbg_ve.ap()[s, :, 0 : Ts[s]], ve[s][:])

            # ---- attention (slot s == mask class s) ----
            aT = [
                qkpool.tile([128, S], bf16, tag=f"at{s}", name=f"at{s}")
                for s in range(NSLOT)
            ]
            for s in range(NSLOT):
                T = Ts[s]
                for qb in range(NQ):
                    qsl = slice(qb * QB, (qb + 1) * QB)
                    p = xpool.tile([128, 2, T, QB], bf16, tag="x", name="p")
                    for kt in range(T):
                        ksl = slice(kt * KT, (kt + 1) * KT)
                        ss = pss.tile([128, 2, QB], fp32, tag="s", name="ss")
                        # scores^T, 2 same-class heads packed as PE row tiles
                        nc.tensor.matmul(ss[:, 0, :], kts[s][0:64, ksl], qts[s][0:64, qsl])
                        nc.tensor.matmul(
                            ss[:, 1, :], kts[s][64:128, ksl], qts[s][64:128, qsl]
                        )
                        bias_ap = bias_sb[:, s : s + 1] if kt == T - 1 else 0.0
                        nc.scalar.activation(
                            p[:, :, kt, :], ss[:], EXP, bias=bias_ap, scale=0.125
                        )
                    if taps and s == 0:
                        if qb == 0:
                            nc.sync.dma_start(dbg_p.ap()[:, :, 0:T, :], p[:])
                    pv = [
                        pspv.tile([128, QB], fp32, tag="pv", name=f"pv{h}")
                        for h in range(2)
                    ]
                    for h in range(2):
                        for kt in range(T):
                            nc.tensor.matmul(
                                pv[h][0 : HD + 1, :],
                                ve[s][:, kt, h, :],
                                p[:, h, kt, :],
                                start=(kt == 0),
                                stop=(kt == T - 1),
                            )
                    # normalize: aT[s][h*64:(h+1)*64, qsl] = pv[h][:64] / pv[h][64]
                    # (sum row moved to partition 0 first: reciprocal /
                    #  partition_broadcast only work from base partition 0)
                    s_sb = smpool.tile([HD + 1, 2, QB], fp32, tag="ssb", name="ssb")
                    for h in range(2):
                        nc.vector.tensor_copy(
                            s_sb[HD : HD + 1, h, :], pv[h][HD : HD + 1, :]
                        )
                    s0 = smpool.tile([1, 2, QB], fp32, tag="s0", name="s0")
                    nc.sync.dma_start(s0[:], s_sb[HD : HD + 1, :, :])
                    r0 = smpool.tile([1, 2, QB], fp32, tag="r0", name="r0")
                    nc.vector.reciprocal(r0[:], s0[:])
                    for h in range(2):
                        rb = smpool.tile([HD, QB], fp32, name=f"rb{h}", tag="rb")
                        nc.gpsimd.partition_broadcast(rb[:], r0[0:1, h, :])
                        if h == 0:
                            nc.vector.tensor_mul(
                                aT[s][0:HD, qsl], pv[h][0:HD, :], rb[:]
                            )
                        else:
                            tmp = smpool.tile([HD, QB], bf16, tag="tmp", name="tmp")
                            nc.vector.tensor_mul(tmp[:], pv[h][0:HD, :], rb[:])
                            nc.sync.dma_start(aT[s][HD:128, qsl], tmp[:])

            if taps:
                for s in range(NSLOT):
                    nc.sync.dma_start(dbg_at.ap()[s], aT[s][:])

            # ---- Wo:  out2 = sum_s aT[s].T @ wo[s]  (partial, fp16) ----
            for qt in range(S // 128):
                ob = opool.tile([128, D], fp16, tag="ob", name="ob")
                for nh in range(2):
                    nsl = slice(nh * 512, (nh + 1) * 512)
                    ps = psmm.tile([128, QB], fp32, tag="mm", name="pso")
                    for s in range(NSLOT):
                        nc.tensor.matmul(
                            ps[:],
                            aT[s][:, qt * 128 : (qt + 1) * 128],
                            wo_sb[:, s, nsl],
                            start=(s == 0),
                            stop=(s == NSLOT - 1),
                        )
                    if nh == 0:
                        nc.vector.tensor_copy(ob[:, nsl], ps[:])
                    else:
                        nc.scalar.copy(ob[:, nsl], ps[:])
                nc.sync.dma_start(out2.ap()[qt * 128 : (qt + 1) * 128, :], ob[:])


def build_in_maps(query, key, value, valid_length, Wq, Wk, Wv, Wo):
    """Host-side sharding. Returns (Ts, in_maps)."""
    valid = np.asarray(valid_length).astype(np.int64)
    Ts = tuple(int(-(-v // KT)) for v in valid)
    CKMAX = max(Ts) * KT

    bf = ml_dtypes.bfloat16
    query = np.asarray(query)
    key = np.asarray(key)
    value = np.asarray(value)
    qTs = [np.ascontiguousarray(query[b].T).astype(bf) for b in range(B)]
    kTs = [np.ascontiguousarray(key[b].T[:, :CKMAX]).astype(bf) for b in range(B)]
    vTs = [np.ascontiguousarray(value[b].T[:, :CKMAX]).astype(bf) for b in range(B)]

    bias = np.zeros((KT, NSLOT), np.float32)
    for s in range(NSLOT):
        rem = int(valid[s]) - (Ts[s] - 1) * KT  # 1..128 valid rows in last tile
        bias[rem:, s] = MASK_BIAS

    Wqb = np.asarray(Wq).astype(bf)
    Wkb = np.asarray(Wk).astype(bf)
    Wvb = np.asarray(Wv).astype(bf)
    Wob = np.asarray(Wo).astype(bf)

    in_maps = []
    for c in range(NCORES):
        beta = c % 4
        hcols = np.concatenate(
            [np.arange(h * HD, (h + 1) * HD) for h in core_heads(c)]
        )
        in_maps.append(
            {
                "qT": qTs[beta],
                "kT": kTs[beta],
                "vT": vTs[beta],
                "wq": np.ascontiguousarray(Wqb[:, hcols]),
                "wk": np.ascontiguousarray(Wkb[:, hcols]),
                "wv": np.ascontiguousarray(Wvb[:, hcols]),
                "wo": np.ascontiguousarray(Wob[hcols, :]),
                "bias": bias,
            }
        )
    return Ts, in_maps


def kernel(query, key, value, valid_length, Wq, Wk, Wv, Wo):
    from concourse.bass_utils import run_bass_kernel_spmd

    Ts, in_maps = build_in_maps(
        query, key, value, valid_length, Wq, Wk, Wv, Wo
    )
    if Ts not in _compiled:
        _compiled[Ts] = _build(Ts)
    nc = _compiled[Ts]

    res = run_bass_kernel_spmd(nc, in_maps, list(range(NCORES)))
    out = np.zeros((B, S, D), np.float32)
    for c in range(NCORES):
        out[c % 4] += res.results[c]["out2"].astype(np.float32)
    return out


# revision 27
# speedup vs baseline: 183.1340x; 176.6629x over previous
"""Multi-head attention (B=4, S=2048, D=1024, H=16) on 8 NeuronCores.

Reference quirk: the key-padding mask uses jnp.tile(valid_length, H) indexed
by the flat (b*H + h) head-batch index, so the effective mask length for
(batch b, head h) is valid_length[h % 4] -- it depends on the head CLASS
(h mod 4), not the batch.

Sharding: core i handles batch i%4 and the 8 heads {4P..4P+3, 4P+8..4P+11}
(P = i//4).  Those 8 heads contain each mask class exactly twice, so every
core does identical work (load-balanced by construction), and key/value work
beyond valid_length[class] (rounded up to 128) is skipped entirely.  The two
same-class heads (h, h+8) are row-packed into one 64-contraction PE pair.
Per-core partial outputs (rank-512 contributions through Wo) are summed on
the host (cores i and i+4 hold the two halves of batch i%4's heads).

All matmuls run in bf16 (fp32 PSUM accumulation).  Attention is computed in
"transposed" orientation S^T[k, q] so that softmax masking is a per-partition
exp bias, the k-sum comes free via an appended ones-column on V, and no
on-chip transposes are needed anywhere.
"""

import sys

for _p in ("/opt/trn_rl_repo", "/root/.axon_site/_ro/trn_rl_repo"):
    if _p not in sys.path:
        sys.path.insert(0, _p)

import numpy as np
import ml_dtypes

B, S, D, H = 4, 2048, 1024, 16
HD = D // H  # 64
NCORES = 8
NSLOT = 4  # head classes (h % 4) per core, 2 heads each
KT = 128  # k-tile size
QB = 512  # q block
MASK_BIAS = -30000.0  # exp(s/8 + bias) == 0 for masked rows (s/8 is O(10))

_compiled = {}  # (T0,T1,T2,T3) -> compiled nc


def core_heads(core):
    """The 8 heads of `core`, in (slot, pair) order: [hA0, hB0, hA1, ...]."""
    P = core // 4
    heads = []
    for c in range(NSLOT):
        heads += [c + 4 * P, c + 8 + 4 * P]
    return heads


def _build(Ts, taps=False, bench_iters=0):
    """Build + compile the single SPMD program for k-tile class profile Ts.

    bench_iters > 0 wraps the whole body in a hardware loop for timing.
    """
    import contextlib
    import concourse.bacc as bacc
    import concourse.tile as tile
    import concourse.mybir as mybir

    fp32 = mybir.dt.float32
    bf16 = mybir.dt.bfloat16
    fp16 = mybir.dt.float16
    EXP = mybir.ActivationFunctionType.Exp

    CKMAX = max(Ts) * KT
    DT = D // 128  # 8 contraction tiles for the projections
    NQ = S // QB  # 4 q blocks
    HPC2 = 2 * NSLOT * HD  # 512 head-dim columns per core

    nc = bacc.Bacc("TRN2", target_bir_lowering=False, debug=False, num_devices=NCORES)

    qT = nc.dram_tensor("qT", [D, S], bf16, kind="ExternalInput")
    kT = nc.dram_tensor("kT", [D, CKMAX], bf16, kind="ExternalInput")
    vT = nc.dram_tensor("vT", [D, CKMAX], bf16, kind="ExternalInput")
    wq = nc.dram_tensor("wq", [D, HPC2], bf16, kind="ExternalInput")
    wk = nc.dram_tensor("wk", [D, HPC2], bf16, kind="ExternalInput")
    wv = nc.dram_tensor("wv", [D, HPC2], bf16, kind="ExternalInput")
    wo = nc.dram_tensor("wo", [HPC2, D], bf16, kind="ExternalInput")
    bias_in = nc.dram_tensor("bias", [KT, NSLOT], fp32, kind="ExternalInput")
    out2 = nc.dram_tensor("out2", [S, D], fp16, kind="ExternalOutput")
    if taps:
        dbg_qts = nc.dram_tensor("dbg_qts", [NSLOT, 128, S], bf16, kind="ExternalOutput")
        dbg_kts = nc.dram_tensor("dbg_kts", [NSLOT, 128, CKMAX], bf16, kind="ExternalOutput")
        dbg_ve = nc.dram_tensor(
            "dbg_ve", [NSLOT, 128, max(Ts), 2, HD + 1], bf16, kind="ExternalOutput"
        )
        dbg_p = nc.dram_tensor("dbg_p", [128, max(Ts), 2, QB], bf16, kind="ExternalOutput")
        dbg_at = nc.dram_tensor("dbg_at", [NSLOT, 128, S], bf16, kind="ExternalOutput")

    with tile.TileContext(nc) as tc:
        with (
            tc.tile_pool(name="w", bufs=1) as wpool,
            tc.tile_pool(name="x", bufs=2) as xpool,
            tc.tile_pool(name="qk", bufs=1) as qkpool,
            tc.tile_pool(name="sm", bufs=2) as smpool,
            tc.tile_pool(name="o", bufs=2) as opool,
            tc.tile_pool(name="psmm", bufs=2, space="PSUM") as psmm,
            tc.tile_pool(name="pss", bufs=2, space="PSUM") as pss,
            tc.tile_pool(name="pspv", bufs=2, space="PSUM") as pspv,
        ):
            # ---- persistent weights ----
            wq_sb = wpool.tile([128, DT, HPC2], bf16, tag="wq")
            wk_sb = wpool.tile([128, DT, HPC2], bf16, tag="wk")
            wv_sb = wpool.tile([128, DT, HPC2], bf16, tag="wv")
            wo_sb = wpool.tile([128, NSLOT, D], bf16, tag="wo")
            bias_sb = wpool.tile([KT, NSLOT], fp32, tag="bias")
            nc.sync.dma_start(wq_sb[:], wq.ap().rearrange("(t p) c -> p t c", p=128))
            nc.sync.dma_start(wk_sb[:], wk.ap().rearrange("(t p) c -> p t c", p=128))
            nc.sync.dma_start(wv_sb[:], wv.ap().rearrange("(t p) c -> p t c", p=128))
            nc.sync.dma_start(wo_sb[:], wo.ap().rearrange("(c p) n -> p c n", p=128))
            nc.sync.dma_start(bias_sb[:], bias_in.ap())

            loop_cm = (
                tc.For_i(0, bench_iters, 1)
                if bench_iters > 0
                else contextlib.nullcontext()
            )
            with loop_cm:
                _emit_body(nc, tc, locals())

    nc.compile()
    return nc


def _emit_body(nc, tc, env):
    import concourse.mybir as mybir

    fp32 = mybir.dt.float32
    bf16 = mybir.dt.bfloat16
    fp16 = mybir.dt.float16
    EXP = mybir.ActivationFunctionType.Exp
    Ts, taps = env["Ts"], env["taps"]
    DT, NQ = env["DT"], env["NQ"]
    CKMAX = env["CKMAX"]
    qT, kT, vT, out2 = env["qT"], env["kT"], env["vT"], env["out2"]
    wq_sb, wk_sb, wv_sb, wo_sb = env["wq_sb"], env["wk_sb"], env["wv_sb"], env["wo_sb"]
    bias_sb = env["bias_sb"]
    xpool, qkpool, smpool, opool = env["xpool"], env["qkpool"], env["smpool"], env["o" "pool"]
    psmm, pss, pspv = env["psmm"], env["pss"], env["pspv"]
    if taps:
        dbg_qts, dbg_kts, dbg_ve = env["dbg_qts"], env["dbg_kts"], env["dbg_ve"]
        dbg_p, dbg_at = env["dbg_p"], env["dbg_at"]

    if True:
        if True:
            # ---- projections (slot s uses weight columns [128s : 128s+128]) ----
            qts = [
                qkpool.tile([128, S], bf16, tag=f"qts{s}", name=f"qts{s}")
                for s in range(NSLOT)
            ]
            xq = xpool.tile([128, DT, S], bf16, tag="x", name="xq")
            nc.sync.dma_start(xq[:], qT.ap().rearrange("(t p) q -> p t q", p=128))
            for s in range(NSLOT):
                csl = slice(s * 128, (s + 1) * 128)
                for qb in range(NQ):
                    ps = psmm.tile([128, QB], fp32, tag="mm", name="psq")
                    for dt in range(DT):
                        nc.tensor.matmul(
                            ps[:],
                            wq_sb[:, dt, csl],
                            xq[:, dt, qb * QB : (qb + 1) * QB],
                            start=(dt == 0),
                            stop=(dt == DT - 1),
                        )
                    nc.vector.tensor_copy(qts[s][:, qb * QB : (qb + 1) * QB], ps[:])

            kts = [
                qkpool.tile([128, Ts[s] * KT], bf16, tag=f"kts{s}", name=f"kts{s}")
                for s in range(NSLOT)
            ]
            xk = xpool.tile([128, DT, CKMAX], bf16, tag="x", name="xk")
            nc.sync.dma_start(xk[:], kT.ap().rearrange("(t p) k -> p t k", p=128))
            for s in range(NSLOT):
                csl = slice(s * 128, (s + 1) * 128)
                CK = Ts[s] * KT
                for k0 in range(0, CK, QB):
                    kw = min(QB, CK - k0)
                    ps = psmm.tile([128, QB], fp32, tag="mm", name="psk")
                    for dt in range(DT):
                        nc.tensor.matmul(
                            ps[:, :kw],
                            wk_sb[:, dt, csl],
                            xk[:, dt, k0 : k0 + kw],
                            start=(dt == 0),
                            stop=(dt == DT - 1),
                        )
                    nc.vector.tensor_copy(kts[s][:, k0 : k0 + kw], ps[:, :kw])

            # V_ext: [128k, T, 2 heads, 65] with ones in column 64
            ve = [
                qkpool.tile([128, Ts[s], 2, HD + 1], bf16, tag=f"ve{s}", name=f"ve{s}")
                for s in range(NSLOT)
            ]
            xv = xpool.tile([128, DT, CKMAX], bf16, tag="x", name="xv")
            nc.sync.dma_start(xv[:], vT.ap().rearrange("(t p) k -> p t k", p=128))
            for s in range(NSLOT):
                csl = slice(s * 128, (s + 1) * 128)
                for kt in range(Ts[s]):
                    nc.gpsimd.memset(ve[s][:, kt, :, HD : HD + 1], 1.0)
                    ps = psmm.tile([128, QB], fp32, tag="mm", name="psv")
                    for dt in range(DT):
                        nc.tensor.matmul(
                            ps[:, 0:128],
                            xv[:, dt, kt * KT : (kt + 1) * KT],
                            wv_sb[:, dt, csl],
                            start=(dt == 0),
                            stop=(dt == DT - 1),
                        )
                    nc.vector.tensor_copy(
                        ve[s][:, kt, :, 0:HD],
                        ps[:, 0:128].rearrange("p (h d) -> p h d", h=2),
                    )

            if taps:
                for s in range(NSLOT):
                    nc.sync.dma_start(dbg_qts.ap()[s], qts[s][:])
                    nc.sync.dma_start(dbg_kts.ap()[s, :, 0 : Ts[s] * KT], kts[s][:])
                    nc.sync.dma_start(dbg_ve.ap()[s, :, 0 : Ts[s]], ve[s][:])

            # ---- attention (slot s == mask class s) ----
            aT = [
                qkpool.tile([128, S], bf16, tag=f"at{s}", name=f"at{s}")
                for s in range(NSLOT)
            ]
            for s in range(NSLOT):
                T = Ts[s]
                for qb in range(NQ):
                    qsl = slice(qb * QB, (qb + 1) * QB)
                    p = xpool.tile([128, T, 2, QB], bf16, tag="x", name="p")
                    for kt in range(T):
                        ksl = slice(kt * KT, (kt + 1) * KT)
                        ss = pss.tile([128, 2, QB], fp32, tag="s", name="ss")
                        # scores^T, 2 same-class heads packed as PE row tiles
                        nc.tensor.matmul(ss[:, 0, :], kts[s][0:64, ksl], qts[s][0:64, qsl])
                        nc.tensor.matmul(
                            ss[:, 1, :], kts[s][64:128, ksl], qts[s][64:128, qsl]
                        )
                        bias_ap = bias_sb[:, s : s + 1] if kt == T - 1 else 0.0
                        # contiguous [128, 1024] exp write (kt-major P layout)
                        nc.scalar.activation(
                            p[:, kt, :, :], ss[:], EXP, bias=bias_ap, scale=0.125
                        )
                    if taps and s == 0:
                        if qb == 0:
                            nc.sync.dma_start(dbg_p.ap()[:, 0:T, :, :], p[:])
                    pv = [
                        pspv.tile([128, QB], fp32, tag="pv", name=f"pv{h}")
                        for h in range(2)
                    ]
                    for h in range(2):
                        for kt in range(T):
                            nc.tensor.matmul(
                                pv[h][0 : HD + 1, :],
                                ve[s][:, kt, h, :],
                                p[:, kt, h, :],
                                start=(kt == 0),
                                stop=(kt == T - 1),
                            )
                    # normalize: aT[s][h*64:(h+1)*64, qsl] = pv[h][:64] / pv[h][64]
                    # (sum row moved to partition 0 first: reciprocal /
                    #  partition_broadcast only work from base partition 0)
                    s_sb = smpool.tile([HD + 1, 2, QB], fp32, tag="ssb", name="ssb")
                    for h in range(2):
                        nc.vector.tensor_copy(
                            s_sb[HD : HD + 1, h, :], pv[h][HD : HD + 1, :]
                        )
                    s0 = smpool.tile([1, 2, QB], fp32, tag="s0", name="s0")
                    nc.sync.dma_start(s0[:], s_sb[HD : HD + 1, :, :])
                    r0 = smpool.tile([1, 2, QB], fp32, tag="r0", name="r0")
                    nc.vector.reciprocal(r0[:], s0[:])
                    for h in range(2):
                        rb = smpool.tile([HD, QB], fp32, name=f"rb{h}", tag="rb")
                        nc.gpsimd.partition_broadcast(rb[:], r0[0:1, h, :])
                        if h == 0:
                            nc.vector.tensor_mul(
                                aT[s][0:HD, qsl], pv[h][0:HD, :], rb[:]
                            )
                        else:
                            tmp = smpool.tile([HD, QB], bf16, tag="tmp", name="tmp")
                            nc.vector.tensor_mul(tmp[:], pv[h][0:HD, :], rb[:])
                            nc.sync.dma_start(aT[s][HD:128, qsl], tmp[:])

            if taps:
                for s in range(NSLOT):
                    nc.sync.dma_start(dbg_at.ap()[s], aT[s][:])

            # ---- Wo:  out2 = sum_s aT[s].T @ wo[s]  (partial, fp16) ----
            for qt in range(S // 128):
                ob = opool.tile([128, D], fp16, tag="ob", name="ob")
                for nh in range(2):
                    nsl = slice(nh * 512, (nh + 1) * 512)
                    ps = psmm.tile([128, QB], fp32, tag="mm", name="pso")
                    for s in range(NSLOT):
                        nc.tensor.matmul(
                            ps[:],
                            aT[s][:, qt * 128 : (qt + 1) * 128],
                            wo_sb[:, s, nsl],
                            start=(s == 0),
                            stop=(s == NSLOT - 1),
                        )
                    if nh == 0:
                        nc.vector.tensor_copy(ob[:, nsl], ps[:])
                    else:
                        nc.scalar.copy(ob[:, nsl], ps[:])
                nc.sync.dma_start(out2.ap()[qt * 128 : (qt + 1) * 128, :], ob[:])


def build_in_maps(query, key, value, valid_length, Wq, Wk, Wv, Wo):
    """Host-side sharding. Returns (Ts, in_maps)."""
    valid = np.asarray(valid_length).astype(np.int64)
    Ts = tuple(int(-(-v // KT)) for v in valid)
    CKMAX = max(Ts) * KT

    bf = ml_dtypes.bfloat16
    query = np.asarray(query)
    key = np.asarray(key)
    value = np.asarray(value)
    qTs = [np.ascontiguousarray(query[b].T).astype(bf) for b in range(B)]
    kTs = [np.ascontiguousarray(key[b].T[:, :CKMAX]).astype(bf) for b in range(B)]
    vTs = [np.ascontiguousarray(value[b].T[:, :CKMAX]).astype(bf) for b in range(B)]

    bias = np.zeros((KT, NSLOT), np.float32)
    for s in range(NSLOT):
        rem = int(valid[s]) - (Ts[s] - 1) * KT  # 1..128 valid rows in last tile
        bias[rem:, s] = MASK_BIAS

    Wqb = np.asarray(Wq).astype(bf)
    Wkb = np.asarray(Wk).astype(bf)
    Wvb = np.asarray(Wv).astype(bf)
    Wob = np.asarray(Wo).astype(bf)

    in_maps = []
    for c in range(NCORES):
        beta = c % 4
        hcols = np.concatenate(
            [np.arange(h * HD, (h + 1) * HD) for h in core_heads(c)]
        )
        in_maps.append(
            {
                "qT": qTs[beta],
                "kT": kTs[beta],
                "vT": vTs[beta],
                "wq": np.ascontiguousarray(Wqb[:, hcols]),
                "wk": np.ascontiguousarray(Wkb[:, hcols]),
                "wv": np.ascontiguousarray(Wvb[:, hcols]),
                "wo": np.ascontiguousarray(Wob[hcols, :]),
                "bias": bias,
            }
        )
    return Ts, in_maps


def kernel(query, key, value, valid_length, Wq, Wk, Wv, Wo):
    from concourse.bass_utils import run_bass_kernel_spmd

    Ts, in_maps = build_in_maps(
        query, key, value, valid_length, Wq, Wk, Wv, Wo
    )
    if Ts not in _compiled:
        _compiled[Ts] = _build(Ts)
    nc = _compiled[Ts]

    res = run_bass_kernel_spmd(nc, in_maps, list(range(NCORES)))
    out = np.zeros((B, S, D), np.float32)
    for c in range(NCORES):
        out[c % 4] += res.results[c]["out2"].astype(np.float32)
    return out
